# revision 21
# baseline (speedup 1.0000x reference)
"""ArcLengthLoss distributed Trainium2 kernel (8 NeuronCores, batch-parallel).

loss = 0.5 * mean(exp(-x[:, adj])) + 0.5 * mean(|x @ M.T|)   over the batch,
with x = output[..., 0] of shape [500000, 28], M the fixed [219, 28] subset-
consistency matrix, adj = columns 21..27.

Strategy: pure data parallel over the batch.  kernel() pre-shards and
pre-transposes each core's rows into a matmul-ready layout on the host
(pair dim on partitions, 32-padded, batch subsets interleaved mod 4), the
SPMD Bass kernel computes per-core partial sums (TensorE K=28 row-group-
packed matmuls -> fused abs+sum evacuation split across DVE and Act;
Act exp for the adjacent-pair term), and the host combines 8 tiny [128, 4]
partials into the scalar loss in float64.
"""

import itertools
import sys

import numpy as np

if "/opt/trn_rl_repo" not in sys.path:
    sys.path.insert(0, "/opt/trn_rl_repo")

import concourse.bass as bass  # noqa: E402
import concourse.tile as tile  # noqa: E402
from concourse import bacc, mybir  # noqa: E402
from concourse.bass_utils import run_bass_kernel_spmd  # noqa: E402

N_CORES = 8
NPAIRS = 28
NB = 219        # rows of M
N_A = 7         # adjacent pairs = columns 21..27 of x
MACRO = 2048    # batch rows per macro tile
FCOLS = MACRO // 4
ALPHA = 0.5

TRACE = False   # test harness sets True to get exec_time_ns


def _build_m_matrix(n=8):
    idx = {}
    s = 0
    for gap in range(2, n):
        for j in range(n - gap):
            idx[(j, j + gap)] = s
            s += 1
    for j in range(n - 1):
        idx[(j, j + 1)] = s
        s += 1
    npairs = n * (n - 1) // 2
    rows = []
    for size in range(3, n + 1):
        for ep in itertools.combinations(range(n), size):
            row = np.zeros(npairs, dtype=np.float32)
            row[idx[(ep[0], ep[-1])]] += 1.0
            for a, b in zip(ep, ep[1:]):
                row[idx[(a, b)]] -= 1.0
            rows.append(row)
    return np.stack(rows)  # [219, 28]


_M = _build_m_matrix()


def _mt_replicated():
    out = np.zeros((128, NB), dtype=np.float32)
    for g in range(4):
        out[32 * g : 32 * g + NPAIRS, :] = _M.T
    return out


def _prep_shard(x_shard, rows):
    """x_shard [n, 28] fp32 -> (xt [128, rows/4], xa [128, rows/128, 7])."""
    n = x_shard.shape[0]
    x = x_shard
    if n < rows:
        x = np.zeros((rows, NPAIRS), dtype=np.float32)
        x[:n] = x_shard
    x4 = x.reshape(rows // 4, 4, NPAIRS).transpose(1, 2, 0)
    xt = np.zeros((4, 32, rows // 4), dtype=np.float32)
    xt[:, :NPAIRS, :] = x4
    xt = np.ascontiguousarray(xt.reshape(128, rows // 4))
    xa = np.ascontiguousarray(x[:, 21:28].reshape(128, rows // 128, N_A))
    return xt, xa


def _build_kernel(rows):
    assert rows % MACRO == 0
    n_macro = rows // MACRO
    n_sb = 2 * n_macro
    f32 = mybir.dt.float32
    bf16 = mybir.dt.bfloat16

    nc = bacc.Bacc(trn_type="TRN2", num_swdge_queues=4)
    xt_ext = nc.declare_dram_parameter("xt", [128, rows // 4], f32, isOutput=False)
    xa_ext = nc.declare_dram_parameter(
        "xa", [128, rows // 128, N_A], f32, isOutput=False
    )
    mt_ext = nc.declare_dram_parameter("mt", [128, NB], f32, isOutput=False)
    out_ext = nc.declare_dram_parameter("out", [128, 4], f32, isOutput=True)

    with tile.TileContext(nc) as tc:
        with (
            tc.tile_pool(name="const", bufs=1) as constp,
            tc.tile_pool(name="xa", bufs=1) as xap,
            tc.tile_pool(name="xin", bufs=2) as xinp,
            tc.tile_pool(name="xbf", bufs=8) as xbfp,
            tc.tile_pool(name="scratch", bufs=2) as scrp,
            tc.tile_pool(name="acc", bufs=1) as accp,
            tc.tile_pool(name="psum", bufs=2, space="PSUM") as psump,
        ):
            mt_f32 = constp.tile([128, NB], f32, tag="mtf")
            nc.sync.dma_start(mt_f32[:], mt_ext[:])
            mt = constp.tile([128, NB], bf16, tag="mtb")
            nc.vector.tensor_copy(mt[:], mt_f32[:])

            acc_a = accp.tile([128, 1], f32, tag="acca")
            acc_bd = accp.tile([128, 2 * n_sb], f32, tag="accbd")
            acc_ba = accp.tile([128, n_sb], f32, tag="accba")

            # A-part data prefetched up front; the exp itself is emitted after
            # the main loop so it fills the pipeline-drain tail on Act.
            xa_f32 = xap.tile([128, rows // 128, N_A], f32, tag="xaf32")
            nc.sync.dma_start(xa_f32[:], xa_ext[:])

            dve_c = 0
            act_c = 0
            for m in range(n_macro):
                # fp32 -> bf16 cast happens inside the (SWDGE) load
                xbf = xbfp.tile([128, FCOLS], bf16, tag="xbf")
                nc.gpsimd.dma_start(xbf[:], xt_ext[:, m * FCOLS : (m + 1) * FCOLS])

                for sb in range(2):
                    # two col-blocks share each PSUM bank (2*219 <= 512);
                    # DVE evacuates banks of groups 0/1, Act groups 2/3.
                    ps_d = psump.tile([128, 2, 512], f32, tag="psd")
                    ps_a = psump.tile([128, 2, 512], f32, tag="psa")
                    for g in range(4):
                        ps = ps_d if g < 2 else ps_a
                        for b2 in range(2):
                            b = 2 * sb + b2
                            nc.tensor.matmul(
                                ps[:, g % 2, NB * b2 : NB * (b2 + 1)],
                                xbf[32 * g : 32 * g + NPAIRS, 128 * b : 128 * (b + 1)],
                                mt[32 * g : 32 * g + NPAIRS, :],
                                tile_position=(32 * g, 0),
                            )
                        if g == 1:
                            nc.vector.tensor_reduce(
                                acc_bd[:, dve_c : dve_c + 2],
                                ps_d[:, :, 0 : 2 * NB],
                                axis=mybir.AxisListType.X,
                                op=mybir.AluOpType.add,
                                apply_absolute_value=True,
                            )
                            dve_c += 2
                    abs_scr = scrp.tile([128, 2, 2 * NB], bf16, tag="abss")
                    nc.scalar.activation(
                        abs_scr[:],
                        ps_a[:, :, 0 : 2 * NB],
                        mybir.ActivationFunctionType.Abs,
                        accum_out=acc_ba[:, act_c : act_c + 1],
                    )
                    act_c += 1

            exp_scr = scrp.tile([128, rows // 128, N_A], f32, tag="exps")
            nc.scalar.activation(
                exp_scr[:],
                xa_f32[:],
                mybir.ActivationFunctionType.Exp,
                scale=-1.0,
                accum_out=acc_a[:, 0:1],
            )

            out_t = accp.tile([128, 4], f32, tag="outt")
            nc.vector.tensor_copy(out_t[:, 0:1], acc_a[:])
            nc.vector.tensor_reduce(
                out_t[:, 1:2], acc_bd[:], axis=mybir.AxisListType.X,
                op=mybir.AluOpType.add,
            )
            nc.vector.tensor_reduce(
                out_t[:, 2:3], acc_ba[:], axis=mybir.AxisListType.X,
                op=mybir.AluOpType.add,
            )
            nc.vector.memset(out_t[:, 3:4], 0.0)
            nc.sync.dma_start(out_ext[:], out_t[:])

    nc.compile()
    return nc


_CACHE = {}


def _get_kernel(rows):
    nc = _CACHE.get(rows)
    if nc is None:
        nc = _build_kernel(rows)
        _CACHE[rows] = nc
    return nc


def kernel(output):
    x = np.asarray(output, dtype=np.float32)
    batch = x.shape[0]
    x = np.ascontiguousarray(x.reshape(batch, NPAIRS))

    per_core = (batch + N_CORES - 1) // N_CORES
    rows = ((per_core + MACRO - 1) // MACRO) * MACRO
    nc = _get_kernel(rows)

    mt = _mt_replicated()
    in_maps = []
    n_pad_total = N_CORES * rows - batch
    for i in range(N_CORES):
        xt, xa = _prep_shard(x[i * per_core : min((i + 1) * per_core, batch)], rows)
        in_maps.append({"xt": xt, "xa": xa, "mt": mt})

    res = run_bass_kernel_spmd(
        nc, in_maps, core_ids=list(range(N_CORES)), trace=TRACE
    )

    a_sum = 0.0
    b_sum = 0.0
    for i in range(N_CORES):
        r = res.results[i]["out"]
        a_sum += np.sum(r[:, 0].astype(np.float64))
        b_sum += np.sum(r[:, 1].astype(np.float64) + r[:, 2].astype(np.float64))
    a_sum -= float(N_A) * n_pad_total  # zero pad rows contribute exp(0)=1 each
    a_mean = a_sum / (N_A * batch)
    b_mean = b_sum / (NB * batch)
    loss = np.float32(ALPHA * a_mean + (1.0 - ALPHA) * b_mean)

    kernel.last_exec_time_ns = res.exec_time_ns
    return np.asarray(loss, dtype=np.float32)


kernel.last_exec_time_ns = None


# revision 22
# speedup vs baseline: 1.2011x; 1.2011x over previous
"""ArcLengthLoss distributed Trainium2 kernel (8 NeuronCores, batch-parallel).

loss = 0.5 * mean(exp(-x[:, adj])) + 0.5 * mean(|x @ M.T|)   over the batch,
with x = output[..., 0] of shape [500000, 28], M the fixed [219, 28] subset-
consistency matrix, adj = columns 21..27.

Strategy: pure data parallel over the batch.  kernel() pre-shards and
pre-transposes each core's rows into a matmul-ready layout on the host
(pair dim on partitions, 32-padded, batch subsets interleaved mod 4), the
SPMD Bass kernel computes per-core partial sums (TensorE K=28 row-group-
packed matmuls -> fused abs+sum evacuation split across DVE and Act;
Act exp for the adjacent-pair term), and the host combines 8 tiny [128, 4]
partials into the scalar loss in float64.
"""

import itertools
import sys

import numpy as np

if "/opt/trn_rl_repo" not in sys.path:
    sys.path.insert(0, "/opt/trn_rl_repo")

import concourse.bass as bass  # noqa: E402
import concourse.tile as tile  # noqa: E402
from concourse import bacc, mybir  # noqa: E402
from concourse.bass_utils import run_bass_kernel_spmd  # noqa: E402

N_CORES = 8
NPAIRS = 28
NB = 219        # rows of M
N_A = 7         # adjacent pairs = columns 21..27 of x
MACRO = 2048    # batch rows per macro tile
FCOLS = MACRO // 4
ALPHA = 0.5

TRACE = False   # test harness sets True to get exec_time_ns


def _build_m_matrix(n=8):
    idx = {}
    s = 0
    for gap in range(2, n):
        for j in range(n - gap):
            idx[(j, j + gap)] = s
            s += 1
    for j in range(n - 1):
        idx[(j, j + 1)] = s
        s += 1
    npairs = n * (n - 1) // 2
    rows = []
    for size in range(3, n + 1):
        for ep in itertools.combinations(range(n), size):
            row = np.zeros(npairs, dtype=np.float32)
            row[idx[(ep[0], ep[-1])]] += 1.0
            for a, b in zip(ep, ep[1:]):
                row[idx[(a, b)]] -= 1.0
            rows.append(row)
    return np.stack(rows)  # [219, 28]


_M = _build_m_matrix()


def _mt_replicated():
    out = np.zeros((128, NB), dtype=np.float32)
    for g in range(4):
        out[32 * g : 32 * g + NPAIRS, :] = _M.T
    return out


def _prep_shard(x_shard, rows):
    """x_shard [n, 28] fp32 -> (xt [128, rows/4], xa [128, rows/128, 7])."""
    n = x_shard.shape[0]
    x = x_shard
    if n < rows:
        x = np.zeros((rows, NPAIRS), dtype=np.float32)
        x[:n] = x_shard
    x4 = x.reshape(rows // 4, 4, NPAIRS).transpose(1, 2, 0)
    xt = np.zeros((4, 32, rows // 4), dtype=np.float32)
    xt[:, :NPAIRS, :] = x4
    xt = np.ascontiguousarray(xt.reshape(128, rows // 4))
    xa = np.ascontiguousarray(x[:, 21:28].reshape(128, rows // 128, N_A))
    return xt, xa


def _build_kernel(rows):
    assert rows % MACRO == 0
    n_macro = rows // MACRO
    n_sb = 2 * n_macro
    f32 = mybir.dt.float32
    bf16 = mybir.dt.bfloat16

    nc = bacc.Bacc(trn_type="TRN2")
    xt_ext = nc.declare_dram_parameter("xt", [128, rows // 4], f32, isOutput=False)
    xa_ext = nc.declare_dram_parameter(
        "xa", [128, rows // 128, N_A], f32, isOutput=False
    )
    mt_ext = nc.declare_dram_parameter("mt", [128, NB], f32, isOutput=False)
    out_ext = nc.declare_dram_parameter("out", [128, 4], f32, isOutput=True)

    with tile.TileContext(nc) as tc:
        with (
            tc.tile_pool(name="const", bufs=1) as constp,
            tc.tile_pool(name="xa", bufs=1) as xap,
            tc.tile_pool(name="xin", bufs=2) as xinp,
            tc.tile_pool(name="xbf", bufs=8) as xbfp,
            tc.tile_pool(name="scratch", bufs=2) as scrp,
            tc.tile_pool(name="acc", bufs=1) as accp,
            tc.tile_pool(name="psum", bufs=2, space="PSUM") as psump,
        ):
            mt_f32 = constp.tile([128, NB], f32, tag="mtf")
            nc.sync.dma_start(mt_f32[:], mt_ext[:])
            mt = constp.tile([128, NB], bf16, tag="mtb")
            nc.vector.tensor_copy(mt[:], mt_f32[:])

            acc_a = accp.tile([128, 1], f32, tag="acca")
            acc_bd = accp.tile([128, 2 * n_sb], f32, tag="accbd")
            acc_ba = accp.tile([128, n_sb], f32, tag="accba")

            # A-part data prefetched up front; the exp itself is emitted after
            # the main loop so it fills the pipeline-drain tail on Act.
            xa_f32 = xap.tile([128, rows // 128, N_A], f32, tag="xaf32")
            nc.sync.dma_start(xa_f32[:], xa_ext[:])

            dve_c = 0
            act_c = 0
            for m in range(n_macro):
                # fp32 -> bf16 cast happens inside the (SWDGE) load
                xbf = xbfp.tile([128, FCOLS], bf16, tag="xbf")
                nc.gpsimd.dma_start(xbf[:], xt_ext[:, m * FCOLS : (m + 1) * FCOLS])

                for sb in range(2):
                    # two col-blocks share each PSUM bank (2*219 <= 512);
                    # DVE evacuates banks of groups 0/1, Act groups 2/3.
                    ps_d = psump.tile([128, 2, 512], f32, tag="psd")
                    ps_a = psump.tile([128, 2, 512], f32, tag="psa")
                    for g in range(4):
                        ps = ps_d if g < 2 else ps_a
                        for b2 in range(2):
                            b = 2 * sb + b2
                            nc.tensor.matmul(
                                ps[:, g % 2, NB * b2 : NB * (b2 + 1)],
                                xbf[32 * g : 32 * g + NPAIRS, 128 * b : 128 * (b + 1)],
                                mt[32 * g : 32 * g + NPAIRS, :],
                                tile_position=(32 * g, 0),
                            )
                        if g == 1:
                            nc.vector.tensor_reduce(
                                acc_bd[:, dve_c : dve_c + 2],
                                ps_d[:, :, 0 : 2 * NB],
                                axis=mybir.AxisListType.X,
                                op=mybir.AluOpType.add,
                                apply_absolute_value=True,
                            )
                            dve_c += 2
                    abs_scr = scrp.tile([128, 2, 2 * NB], bf16, tag="abss")
                    nc.scalar.activation(
                        abs_scr[:],
                        ps_a[:, :, 0 : 2 * NB],
                        mybir.ActivationFunctionType.Abs,
                        accum_out=acc_ba[:, act_c : act_c + 1],
                    )
                    act_c += 1

            exp_scr = scrp.tile([128, rows // 128, N_A], f32, tag="exps")
            nc.scalar.activation(
                exp_scr[:],
                xa_f32[:],
                mybir.ActivationFunctionType.Exp,
                scale=-1.0,
                accum_out=acc_a[:, 0:1],
            )

            out_t = accp.tile([128, 4], f32, tag="outt")
            nc.vector.tensor_copy(out_t[:, 0:1], acc_a[:])
            nc.vector.tensor_reduce(
                out_t[:, 1:2], acc_bd[:], axis=mybir.AxisListType.X,
                op=mybir.AluOpType.add,
            )
            nc.vector.tensor_reduce(
                out_t[:, 2:3], acc_ba[:], axis=mybir.AxisListType.X,
                op=mybir.AluOpType.add,
            )
            nc.vector.memset(out_t[:, 3:4], 0.0)
            nc.sync.dma_start(out_ext[:], out_t[:])

    nc.compile()
    return nc


_CACHE = {}


def _get_kernel(rows):
    nc = _CACHE.get(rows)
    if nc is None:
        nc = _build_kernel(rows)
        _CACHE[rows] = nc
    return nc


def kernel(output):
    x = np.asarray(output, dtype=np.float32)
    batch = x.shape[0]
    x = np.ascontiguousarray(x.reshape(batch, NPAIRS))

    per_core = (batch + N_CORES - 1) // N_CORES
    rows = ((per_core + MACRO - 1) // MACRO) * MACRO
    nc = _get_kernel(rows)

    mt = _mt_replicated()
    in_maps = []
    n_pad_total = N_CORES * rows - batch
    for i in range(N_CORES):
        xt, xa = _prep_shard(x[i * per_core : min((i + 1) * per_core, batch)], rows)
        in_maps.append({"xt": xt, "xa": xa, "mt": mt})

    res = run_bass_kernel_spmd(
        nc, in_maps, core_ids=list(range(N_CORES)), trace=TRACE
    )

    a_sum = 0.0
    b_sum = 0.0
    for i in range(N_CORES):
        r = res.results[i]["out"]
        a_sum += np.sum(r[:, 0].astype(np.float64))
        b_sum += np.sum(r[:, 1].astype(np.float64) + r[:, 2].astype(np.float64))
    a_sum -= float(N_A) * n_pad_total  # zero pad rows contribute exp(0)=1 each
    a_mean = a_sum / (N_A * batch)
    b_mean = b_sum / (NB * batch)
    loss = np.float32(ALPHA * a_mean + (1.0 - ALPHA) * b_mean)

    kernel.last_exec_time_ns = res.exec_time_ns
    return np.asarray(loss, dtype=np.float32)


kernel.last_exec_time_ns = None
